# revision 20
# baseline (speedup 1.0000x reference)
"""Trainium2 Bass kernel for BallUnpooling (gnn_message_passing).

Math per parent n (N parents, 8 children each):
  rel_dist[n,m] = |children_pos[8n+m] - pos[n]|                       [N,8]
  sc_cat = [sc | rel_dist]                                            [N,40]
  mv_p[n]  = W_mv @ mv[n]            (128x32 @ 32x16 -> 128x16)
  mv_p[n][:,0] += W_s2mv @ sc_cat[n]
  sc_p[n]  = W_s @ sc_cat[n] + W_mv2s @ mv[n][:,0]                    [128]
  c_mv = children_mv + mv_p   -> per-child RMS-norm over (16,16)
  c_sc = children_sc + sc_p   -> per-child LayerNorm over 16

Sharding: data-parallel over parents, 8192 parents/core on 8 cores.

Layout: the mv path runs in "block layout": partition = o = (child m, channel
d) = 128, free = (parent n, component e). children_mv is loaded / c_mv stored
directly in this layout (DMA innermost chunk = 64B). Per-child RMS stats are
computed with a block-diagonal-ones matmul over the partition groups plus a
grouped free-dim reduce. The scalar path runs in parent-row layout
(partition = parent), with one PE transpose carrying sc_p across.
"""

from contextlib import ExitStack

import numpy as np

import concourse.bacc as bacc
import concourse.bass as bass
import concourse.tile as tile
from concourse import masks, mybir

F32 = mybir.dt.float32
AF = mybir.ActivationFunctionType
OP = mybir.AluOpType
AX = mybir.AxisListType

N = 65536
STRIDE = 8
IN_DIM = 32
OUT_DIM = 16
MV = 16
EPS = 1e-6
NCORES = 8
NP = N // NCORES          # parents per core
SB = 128                  # parents per superbatch
GRP = 32                  # parents per matmul group (psum free = 32*16 = 512)
NGRP = SB // GRP
ROW = STRIDE * OUT_DIM * MV   # 2048 floats per parent of children_mv
SROW = STRIDE * OUT_DIM       # 128 floats per parent of children_sc
O = STRIDE * OUT_DIM          # 128 output rows (m,d) of the EquiLinear
S_IN = IN_DIM + STRIDE        # 40
GCOL = GRP * MV               # 512


def make_block_ones(nc, ones8, ones8T):
    """ones8[k, m] = 1 if k//16 == m else 0 (128x8); ones8T its transpose."""
    nc.gpsimd.memset(ones8, 1.0)
    # keep where k - 16m >= 0
    nc.gpsimd.affine_select(out=ones8, in_=ones8, fill=0.0, base=0,
                            compare_op=OP.is_ge, channel_multiplier=1,
                            pattern=[[-OUT_DIM, STRIDE]])
    # keep where 16m - k + 15 >= 0
    nc.gpsimd.affine_select(out=ones8, in_=ones8, fill=0.0, base=OUT_DIM - 1,
                            compare_op=OP.is_ge, channel_multiplier=-1,
                            pattern=[[OUT_DIM, STRIDE]])
    nc.gpsimd.memset(ones8T, 1.0)
    nc.gpsimd.affine_select(out=ones8T, in_=ones8T, fill=0.0, base=0,
                            compare_op=OP.is_ge, channel_multiplier=-OUT_DIM,
                            pattern=[[1, O]])
    nc.gpsimd.affine_select(out=ones8T, in_=ones8T, fill=0.0, base=OUT_DIM - 1,
                            compare_op=OP.is_ge, channel_multiplier=OUT_DIM,
                            pattern=[[-1, O]])


def build_kernel(n_parents=NP):
    nc = bacc.Bacc("TRN2", target_bir_lowering=False, debug=False,
                   enable_asserts=False)
    n_child = n_parents * STRIDE

    mv = nc.dram_tensor("mv", (n_parents, IN_DIM, MV), F32, kind="ExternalInput").ap()
    sc = nc.dram_tensor("sc", (n_parents, IN_DIM), F32, kind="ExternalInput").ap()
    pos = nc.dram_tensor("pos", (n_parents, 3), F32, kind="ExternalInput").ap()
    cmv = nc.dram_tensor("children_mv", (n_child, OUT_DIM, MV), F32, kind="ExternalInput").ap()
    csc = nc.dram_tensor("children_sc", (n_child, OUT_DIM), F32, kind="ExternalInput").ap()
    cpos = nc.dram_tensor("children_pos", (n_child, 3), F32, kind="ExternalInput").ap()
    w_mv = nc.dram_tensor("W_mv", (O, IN_DIM), F32, kind="ExternalInput").ap()
    w_s2mv = nc.dram_tensor("W_s2mv", (O, S_IN), F32, kind="ExternalInput").ap()
    w_s = nc.dram_tensor("W_s", (O, S_IN), F32, kind="ExternalInput").ap()
    w_mv2s = nc.dram_tensor("W_mv2s", (O, IN_DIM), F32, kind="ExternalInput").ap()

    out_mv = nc.dram_tensor("c_mv", (n_child, OUT_DIM, MV), F32, kind="ExternalOutput").ap()
    out_sc = nc.dram_tensor("c_sc", (n_child, OUT_DIM), F32, kind="ExternalOutput").ap()

    # block-layout views: [(m d) = 128, (n e)] per superbatch
    cmv_blk = cmv.rearrange("(n m) d e -> n (m d) e", m=STRIDE)
    omv_blk = out_mv.rearrange("(n m) d e -> n (m d) e", m=STRIDE)
    # parent-row views of the scalar tensors
    csc_rows = csc.rearrange("(n m) d -> n (m d)", m=STRIDE)
    cpos_rows = cpos.rearrange("(n m) k -> n (m k)", m=STRIDE)
    osc_rows = out_sc.rearrange("(n m) d -> n (m d)", m=STRIDE)

    with tile.TileContext(nc) as tc, ExitStack() as ctx:
        const = ctx.enter_context(tc.tile_pool(name="const", bufs=1))
        big = ctx.enter_context(tc.tile_pool(name="big", bufs=2))
        med = ctx.enter_context(tc.tile_pool(name="med", bufs=3))
        small = ctx.enter_context(tc.tile_pool(name="small", bufs=4))
        ps_mv = ctx.enter_context(tc.tile_pool(name="ps_mv", bufs=2, space="PSUM"))
        ps_aux = ctx.enter_context(tc.tile_pool(name="ps_aux", bufs=2, space="PSUM"))
        ps_t = ctx.enter_context(tc.tile_pool(name="ps_t", bufs=2, space="PSUM"))
        ps_n = ctx.enter_context(tc.tile_pool(name="ps_n", bufs=2, space="PSUM"))

        ident = const.tile([128, 128], F32)
        masks.make_identity(nc, ident[:])
        eps_t = const.tile([128, 1], F32, tag="eps")
        nc.vector.memset(eps_t[:], EPS)
        ones8 = const.tile([O, STRIDE], F32, tag="ones8")
        ones8T = const.tile([STRIDE, O], F32, tag="ones8T")
        make_block_ones(nc, ones8[:], ones8T[:])

        # ---- one-time weight prep: tiny tensors, DMA them pre-transposed
        def load_wT(name, w_ap, k):
            wt = const.tile([k, O], F32, tag=f"wt_{name}")
            nc.gpsimd.dma_start(out=wt[:], in_=w_ap.rearrange("o k -> k o"))
            return wt

        # 32-row weights replicated at partition bases 0/32 so the lhsT slice
        # shares base_partition with the per-group rhs slice.
        def load_wT_rep(name, w_ap):
            wt = const.tile([64, O], F32, tag=f"wt_{name}")
            for g in range(2):
                nc.gpsimd.dma_start(out=wt[g * IN_DIM:(g + 1) * IN_DIM, :],
                                    in_=w_ap.rearrange("o i -> i o"))
            return wt

        wmvT = load_wT_rep("wmv", w_mv)
        wmv2sT = load_wT_rep("wmv2s", w_mv2s)
        ws2mvT = load_wT("ws2mv", w_s2mv, S_IN)
        wsT = load_wT("ws", w_s, S_IN)

        n_sb = n_parents // SB
        for b in range(n_sb):
            n0 = b * SB

            # ---- children_mv in block layout: one 1MB DMA
            cb = big.tile([O, SB * MV], F32, tag="cb")
            nc.sync.dma_start(out=cb[:], in_=cmv_blk[n0:n0 + SB]
                              .rearrange("n o e -> o n e"))
            # scalar-path loads (parent-row)
            ch_sc = med.tile([SB, SROW], F32, tag="ch_sc")
            nc.sync.dma_start(out=ch_sc[:], in_=csc_rows[n0:n0 + SB])
            ch_pos = small.tile([SB, STRIDE, 3], F32, tag="ch_pos")
            nc.sync.dma_start(out=ch_pos[:], in_=cpos_rows[n0:n0 + SB])
            pos_t = small.tile([SB, 3], F32, tag="pos_t")
            nc.sync.dma_start(out=pos_t[:], in_=pos[n0:n0 + SB])
            geom = small.tile([SB, S_IN], F32, tag="geom")
            nc.sync.dma_start(out=geom[:, 0:IN_DIM], in_=sc[n0:n0 + SB])
            # mv as matmul rhs: two [64, 512] tiles (parents t*64+h*32+n at
            # partition half h); loaded with one DMA per half on two queues
            rhs_tiles = []
            for t in range(NGRP // 2):
                rt = med.tile([64, GCOL], F32, tag=f"rhs_mv{t}")
                for h in range(2):
                    p0 = n0 + t * 2 * GRP + h * GRP
                    eng = nc.sync if h == 0 else nc.scalar
                    eng.dma_start(out=rt[h * GRP:(h + 1) * GRP, :]
                                  .rearrange("i (n e) -> i n e", e=MV),
                                  in_=mv[p0:p0 + GRP].rearrange("n i e -> i n e"))
                rhs_tiles.append(rt)

            # ---- geometry: rel_dist into geom[:, 32:40]
            rel = small.tile([SB, STRIDE, 3], F32, tag="rel")
            nc.vector.tensor_sub(rel[:], ch_pos[:],
                                 pos_t[:, None, :].broadcast_to([SB, STRIDE, 3]))
            nc.vector.tensor_mul(rel[:], rel[:], rel[:])
            d2 = small.tile([SB, STRIDE], F32, tag="d2")
            nc.vector.reduce_sum(d2[:], rel[:], axis=AX.X)
            nc.scalar.sqrt(geom[:, IN_DIM:S_IN], d2[:])

            # ---- transpose sc_cat -> [40, 128]
            pt = ps_t.tile([S_IN, SB], F32, tag="pt")
            nc.tensor.transpose(pt[:], geom[:], ident[:])
            scT = med.tile([S_IN, SB], F32, tag="scT")
            nc.scalar.copy(scT[:], pt[:])

            out_sb = big.tile([O, SB * MV], F32, tag="out_sb")
            scp_cm = med.tile([O, SB], F32, tag="scp_cm")

            for g in range(NGRP):
                t, h = g // 2, g % 2
                gs = slice(g * GRP, (g + 1) * GRP)
                cs = slice(g * GCOL, (g + 1) * GCOL)
                hs = slice(h * GRP, (h + 1) * GRP)
                rhs_g = rhs_tiles[t][hs, :]
                # mv_p block: [128 (m d), (n e)]
                pmv = ps_mv.tile([O, GCOL], F32, tag="pmv")
                nc.tensor.matmul(pmv[:], wmvT[hs, :], rhs_g, start=True, stop=True)
                # aux: [:,0:32] = sc_p ; [:,32:64] = s2mv (grade-0 add)
                paux = ps_aux.tile([O, 2 * GRP], F32, tag="paux")
                nc.tensor.matmul(paux[:, 0:GRP], wsT[:], scT[:, gs],
                                 start=True, stop=False)
                mv0 = rhs_g.rearrange("i (n e) -> i n e", e=MV)[:, :, 0]
                nc.tensor.matmul(paux[:, 0:GRP], wmv2sT[hs, :], mv0,
                                 start=False, stop=True)
                nc.tensor.matmul(paux[:, GRP:2 * GRP], ws2mvT[:], scT[:, gs],
                                 start=True, stop=True)

                # residual add fused with psum read; grade-0 correction
                csum = out_sb[:, cs]
                nc.vector.tensor_add(csum, cb[:, cs], pmv[:])
                c_e0 = csum.rearrange("o (n e) -> o n e", e=MV)[:, :, 0]
                nc.vector.tensor_add(c_e0, c_e0, paux[:, GRP:2 * GRP])
                # sc_p columns into column-major staging
                nc.scalar.copy(scp_cm[:, gs], paux[:, 0:GRP])

                # ---- RMS stats: sum over e (gpsimd), then over d (PE ones)
                sq = med.tile([O, GCOL], F32, tag="sq")
                nc.scalar.square(sq[:], csum)
                esum = small.tile([O, GRP], F32, tag="esum")
                nc.vector.reduce_sum(esum[:], sq[:].rearrange("o (n e) -> o n e", e=MV),
                                     axis=AX.X)
                psum_s = ps_n.tile([STRIDE, GRP], F32, tag="ps_n")
                nc.tensor.matmul(psum_s[:], ones8[:], esum[:], start=True, stop=True)
                rms = small.tile([STRIDE, GRP], F32, tag="rms")
                nc.scalar.activation(rms[:], psum_s[:], AF.Sqrt,
                                     bias=eps_t[0:STRIDE, :], scale=1.0 / 256.0)
                inv = small.tile([STRIDE, GRP], F32, tag="inv")
                nc.vector.reciprocal(inv[:], rms[:])
                psum_e = ps_n.tile([O, GRP], F32, tag="ps_n")
                nc.tensor.matmul(psum_e[:], ones8T[:], inv[:], start=True, stop=True)
                # normalize in place (broadcast inv over e)
                nc.vector.tensor_mul(
                    csum.rearrange("o (n e) -> o n e", e=MV),
                    csum.rearrange("o (n e) -> o n e", e=MV),
                    psum_e[:, :, None].broadcast_to([O, GRP, MV]))

            # ---- store c_mv (block layout)
            nc.sync.dma_start(out=omv_blk[n0:n0 + SB].rearrange("n o e -> o n e"),
                              in_=out_sb[:])

            # ---- scalar path: transpose sc_p to parent rows, add, LayerNorm
            pscp = ps_t.tile([SB, O], F32, tag="pt")
            nc.tensor.transpose(pscp[:], scp_cm[:], ident[:])
            nc.vector.tensor_add(ch_sc[:], ch_sc[:], pscp[:])

            grouped = ch_sc[:].rearrange("p (m d) -> p m d", m=STRIDE)
            s1 = small.tile([SB, STRIDE], F32, tag="s1")
            nc.vector.reduce_sum(s1[:], grouped, axis=AX.X)
            mu = small.tile([SB, STRIDE], F32, tag="mu")
            nc.scalar.mul(mu[:], s1[:], 1.0 / OUT_DIM)
            scsq = med.tile([SB, SROW], F32, tag="scsq")
            nc.scalar.square(scsq[:], ch_sc[:])
            s2 = small.tile([SB, STRIDE], F32, tag="s2")
            nc.vector.reduce_sum(s2[:], scsq[:].rearrange("p (m d) -> p m d", m=STRIDE),
                                 axis=AX.X)
            mu2 = small.tile([SB, STRIDE], F32, tag="mu2")
            nc.vector.tensor_mul(mu2[:], mu[:], mu[:])
            var = small.tile([SB, STRIDE], F32, tag="var")
            nc.vector.tensor_scalar(var[:], s2[:], 1.0 / OUT_DIM, None, op0=OP.mult)
            nc.vector.tensor_sub(var[:], var[:], mu2[:])
            std = small.tile([SB, STRIDE], F32, tag="std")
            nc.scalar.activation(std[:], var[:], AF.Sqrt, bias=eps_t[:])
            inv2 = small.tile([SB, STRIDE], F32, tag="inv2")
            nc.vector.reciprocal(inv2[:], std[:])
            for m in range(STRIDE):
                blk = ch_sc[:, m * OUT_DIM:(m + 1) * OUT_DIM]
                nc.vector.tensor_scalar(blk, blk, mu[:, m:m + 1], inv2[:, m:m + 1],
                                        op0=OP.subtract, op1=OP.mult)

            nc.sync.dma_start(out=osc_rows[n0:n0 + SB], in_=ch_sc[:])

    nc.compile()
    return nc


def kernel(**inputs):
    from concourse.bass_utils import run_bass_kernel_spmd

    nc = build_kernel(NP)
    in_maps = []
    for c in range(NCORES):
        m = {}
        for k in ("mv", "sc", "pos"):
            m[k] = np.ascontiguousarray(inputs[k][c * NP:(c + 1) * NP])
        for k in ("children_mv", "children_sc", "children_pos"):
            m[k] = np.ascontiguousarray(
                inputs[k][c * NP * STRIDE:(c + 1) * NP * STRIDE])
        for k in ("W_mv", "W_s2mv", "W_s", "W_mv2s"):
            m[k] = np.ascontiguousarray(inputs[k])
        in_maps.append(m)

    res = run_bass_kernel_spmd(nc, in_maps, core_ids=list(range(NCORES)))
    c_mv = np.concatenate([r["c_mv"] for r in res.results], axis=0)
    c_sc = np.concatenate([r["c_sc"] for r in res.results], axis=0)
    return c_mv, c_sc
